# revision 1
# baseline (speedup 1.0000x reference)
"""Trainium2 Bass kernel for nn_AttModel (masked GNN attention).

Reference computation (per batch b of 32, N=1024, D=H=O=256):
    v = relu(x @ Wv + bv); q = relu(x @ Wq + bq); k = relu(x @ Wk + bk)
    S = q @ k^T
    att = softmax(S * mask - 9e15 * (1 - mask), axis=-1)
    out = relu((att @ v) @ Wo + bo)

Strategy: pure data parallelism over the batch dim — 8 NeuronCores, 4
batches each, weights replicated, no collectives.  Per core / batch:

  - Host pre-transposes x to x^T [D, N] (bf16) so the D-contraction lands
    on the SBUF partition dim; mask becomes the additive form
    (mask-1)*9e15 in bf16 ({0, -9e15} exactly).
  - Q^T, K^T [H, N] = relu(Wq^T x^T + bq), bf16 matmuls, epilogues split
    ACT/DVE; V [N, H] natural layout (bias via a K=1 ones-row matmul on
    the generic path).
  - S[nsub] [128, 1024] = (Q^T)^T K^T in PSUM; DVE adds the additive
    mask; one ScalarE Exp per 128-row chunk with accum_out giving the
    masked row sums d (masked entries exp to exactly 0).
  - P~ is transposed 128x128-block-wise on the PE (bf16 PSUM) with a
    2-deep software pipeline; AV matmuls accumulate O^T[h, n] per n-half.
  - Y = relu((att @ v) @ Wo + bo) with softmax normalization deferred:
    y = max(Y_psum * (1/d), 0) on DVE ( == relu((O/d) @ Wo + d*bo/d) );
    the d*bo bias row (generic path) comes from a packed PE transpose of
    d and a K=1 row matmul.  When bv == bo == 0 (this problem's inputs)
    the bias machinery is omitted entirely.
  - Emission is phase-interleaved across batches (QKV/S of batch b+1
    between the phases of batch b) so the PE engine FIFO always has
    independent matmul work while ACT/DVE run epilogues.

Measured on HW (8 cores): relative error 4.0e-3 vs the fp32 jax
reference; cost-model (TimelineSim) exec estimate ~127 us/core.
"""

import os

import numpy as np

B, N, DIN, H, DOUT = 32, 1024, 256, 256, 256
NCORES = 8
BP = B // NCORES  # batches per core
P = 128
NSUB = N // P  # 8 row-chunks of 128
NHALF = N // 512  # 2 column-halves of 512

_nc_cache = {}
last_results = None  # BassKernelResults of the most recent run (for test.py)


def _build_nc(bp=BP, zero_bias=False):
    import concourse.mybir as mybir
    import concourse.tile as tile
    from concourse import bacc
    from concourse.masks import make_identity
    from contextlib import ExitStack

    f32 = mybir.dt.float32
    f32r = mybir.dt.float32r
    bf16 = mybir.dt.bfloat16
    AF = mybir.ActivationFunctionType
    ALU = mybir.AluOpType

    nc = bacc.Bacc("TRN2", target_bir_lowering=False)

    xT_d = nc.declare_dram_parameter("xT", [bp, DIN, N], bf16, isOutput=False)
    mask_d = nc.declare_dram_parameter("mask", [bp, N, N], bf16, isOutput=False)
    wq_d = nc.declare_dram_parameter("Wq", [DIN, H], bf16, isOutput=False)
    wk_d = nc.declare_dram_parameter("Wk", [DIN, H], bf16, isOutput=False)
    wv_d = nc.declare_dram_parameter("Wv", [DIN, H], bf16, isOutput=False)
    wo_d = nc.declare_dram_parameter("Wo", [H, DOUT], bf16, isOutput=False)
    bq_d = nc.declare_dram_parameter("bq", [H, 1], f32, isOutput=False)
    bk_d = nc.declare_dram_parameter("bk", [H, 1], f32, isOutput=False)
    bv_d = nc.declare_dram_parameter("bv", [1, H], bf16, isOutput=False)
    bo_d = nc.declare_dram_parameter("bo", [1, DOUT], bf16, isOutput=False)
    ones_d = nc.declare_dram_parameter("ones", [1, P], bf16, isOutput=False)
    out_d = nc.declare_dram_parameter("out", [bp, N, DOUT], f32, isOutput=True)

    with tile.TileContext(nc) as tc, ExitStack() as ctx:
        const = ctx.enter_context(tc.tile_pool(name="const", bufs=1))
        sb = ctx.enter_context(tc.tile_pool(name="sb", bufs=1))
        ps = ctx.enter_context(tc.tile_pool(name="ps", bufs=1, space="PSUM"))

        # ---- constants / weights (loaded once, one DMA per tensor) ----
        # w*_pack[p, c*H + h] = W[c*P + p, h]
        def load_w(d, nm, width):
            t = const.tile([P, 2 * width], bf16, tag=nm, name=nm)
            nc.sync.dma_start(
                t[:].rearrange("p (c h) -> p c h", c=2),
                d.rearrange("(c p) h -> p c h", c=2))
            return [t[:, c * width : (c + 1) * width] for c in range(2)]

        wq_sb = load_w(wq_d, "wqp", H)
        wk_sb = wv_sb = wo_sb = None  # loaded after batch-0 xt DMAs

        def load_b(d, nm):
            t = const.tile([P, 2], f32, tag=nm, name=nm)
            nc.gpsimd.dma_start(
                t[:].rearrange("p (c o) -> p c o", c=2),
                d.rearrange("(c p) o -> p c o", c=2))
            return [t[:, c : c + 1] for c in range(2)]

        bq_sb = load_b(bq_d, "bqp")
        bk_sb = load_b(bk_d, "bkp")
        bv_sb = const.tile([1, H], bf16, tag="bv", name="bv_sb")
        nc.gpsimd.dma_start(bv_sb[:], bv_d[:, :])
        bo_sb = const.tile([1, DOUT], bf16, tag="bo", name="bo_sb")
        nc.gpsimd.dma_start(bo_sb[:], bo_d[:, :])
        ones_row = const.tile([1, P], bf16, tag="ones", name="ones_row")
        nc.gpsimd.dma_start(ones_row[:], ones_d[:, :])
        id_bf = const.tile([P, P], bf16, tag="idbf", name="id_bf")
        make_identity(nc, id_bf[:])
        id_f32 = const.tile([P, P], f32, tag="idf32", name="id_f32")
        make_identity(nc, id_f32[:])

        st = {}

        def qkv_phase(b):
            nonlocal wk_sb, wv_sb, wo_sb
            # ---- load x^T ----
            xt = []
            for c in range(2):
                t = sb.tile([P, N], bf16, tag=f"xt{c}", bufs=3, name=f"xt{c}_{b}")
                nc.sync.dma_start(t[:], xT_d[b, c * P : (c + 1) * P, :])
                xt.append(t)
            if wk_sb is None:
                wk_sb = load_w(wk_d, "wkp", H)
                wv_sb = load_w(wv_d, "wvp", H)
                wo_sb = load_w(wo_d, "wop", DOUT)

            # ---- Q^T, K^T [H, N] (bf16), V [N, H] (bf16) ----
            qt_sb = []
            kt_sb = []
            for hc in range(2):
                qt = sb.tile([P, N], bf16, tag=f"qt{hc}", bufs=3, name=f"qt{hc}_{b}")
                kt = sb.tile([P, N], bf16, tag=f"kt{hc}", bufs=3, name=f"kt{hc}_{b}")
                for nh in range(NHALF):
                    nsl = slice(nh * 512, (nh + 1) * 512)
                    pq = ps.tile([P, 512], f32, tag="s", bufs=3,
                                 name=f"pq{b}_{hc}_{nh}")
                    for dc in range(2):
                        nc.tensor.matmul(
                            pq[:],
                            wq_sb[dc][:, hc * P : (hc + 1) * P],
                            xt[dc][:, nsl],
                            start=(dc == 0),
                            stop=(dc == 1),
                        )
                    nc.scalar.activation(qt[:, nsl], pq[:], AF.Relu,
                                         bias=bq_sb[hc][:])
                    pk = ps.tile([P, 512], f32, tag="s", bufs=3,
                                 name=f"pk{b}_{hc}_{nh}")
                    for dc in range(2):
                        nc.tensor.matmul(
                            pk[:],
                            wk_sb[dc][:, hc * P : (hc + 1) * P],
                            xt[dc][:, nsl],
                            start=(dc == 0),
                            stop=(dc == 1),
                        )
                    nc.vector.tensor_scalar(
                        out=kt[:, nsl], in0=pk[:], scalar1=bk_sb[hc][:],
                        scalar2=0.0, op0=ALU.add, op1=ALU.max,
                    )
                qt_sb.append(qt)
                kt_sb.append(kt)

            v_sb = []
            for mc in range(NSUB):
                msl = slice(mc * P, (mc + 1) * P)
                pv = ps.tile([P, H], f32, tag="s", bufs=3, name=f"pv{b}_{mc}")
                for dc in range(2):
                    nc.tensor.matmul(
                        pv[:], xt[dc][:, msl], wv_sb[dc][:],
                        start=(dc == 0), stop=(zero_bias and dc == 1),
                    )
                if not zero_bias:
                    # + bv via ones-row outer product (K=1)
                    nc.tensor.matmul(pv[:], ones_row[:], bv_sb[:],
                                     start=False, stop=True)
                v = sb.tile([P, H], bf16, tag="v", bufs=16, name=f"v{b}_{mc}")
                nc.vector.tensor_scalar_max(v[:], pv[:], 0.0)
                v_sb.append(v)
            st[b] = {"qt": qt_sb, "kt": kt_sb, "v": v_sb}

        def s_phase(b):
            qt_sb, kt_sb = st[b]["qt"], st[b]["kt"]
            pm_tiles = []
            d_pack = sb.tile([P, NSUB], f32, tag="dp", bufs=2,
                             name=f"dpack{b}")
            for ns in range(NSUB):
                nsl = slice(ns * P, (ns + 1) * P)
                mk = sb.tile([P, N], bf16, tag="mask", bufs=8, name=f"mk{b}_{ns}")
                nc.sync.dma_start(mk[:], mask_d[b, nsl, :])

                sm = sb.tile([P, N], f32, tag="sm", bufs=6, name=f"sm{b}_{ns}")
                for mh in range(NHALF):
                    msl = slice(mh * 512, (mh + 1) * 512)
                    sp = ps.tile([P, 512], f32, tag="s", bufs=3,
                                 name=f"sp{b}_{ns}_{mh}")
                    for hc in range(2):
                        nc.tensor.matmul(
                            sp[:],
                            qt_sb[hc][:, nsl],
                            kt_sb[hc][:, msl],
                            start=(hc == 0),
                            stop=(hc == 1),
                        )
                    # S + maskadd  (maskadd = (mask-1)*9e15, host-precomputed)
                    nc.vector.scalar_tensor_tensor(
                        out=sm[:, msl], in0=sp[:], scalar=1.0, in1=mk[:, msl],
                        op0=ALU.mult, op1=ALU.add,
                    )

                pm = sb.tile([P, N], bf16, tag="pm", bufs=16, name=f"pm{b}_{ns}")
                nc.scalar.activation(pm[:], sm[:], AF.Exp,
                                     accum_out=d_pack[:, ns : ns + 1])
                pm_tiles.append(pm)
            st[b]["pm"] = pm_tiles
            st[b]["dp"] = d_pack

        def trav_phase(b):
            pm_tiles, v_sb = st[b]["pm"], st[b]["v"]
            # ---- transpose P~; accumulate O^T[h, n] one n-half at a time ----
            ptts = {}
            ot_sb = [
                sb.tile([P, N], bf16, tag=f"ot{hc}", bufs=2,
                        name=f"ot{hc}_{b}")
                for hc in range(2)
            ]

            ptps = {}

            def emit_transposes(mc):
                ptp = ps.tile([P, N], bf16, tag="tr", bufs=3,
                              name=f"ptp{b}_{mc}")
                msl = slice(mc * P, (mc + 1) * P)
                for ns in range(NSUB):
                    nc.tensor.transpose(
                        ptp[:, ns * P : (ns + 1) * P],
                        pm_tiles[ns][:, msl],
                        id_bf[:],
                    )
                ptps[mc] = ptp

            for nh in range(NHALF):
                po = [
                    ps.tile([P, 512], f32, tag=f"oh{hc}", bufs=1,
                            name=f"po{b}_{hc}_{nh}")
                    for hc in range(2)
                ]
                if nh == 0:
                    emit_transposes(0)
                    emit_transposes(1)
                for mc in range(NSUB):
                    if nh == 0:
                        if mc + 2 < NSUB:
                            emit_transposes(mc + 2)
                        ptT = sb.tile([P, N], bf16, tag="ptT", bufs=10,
                                      name=f"ptT{b}_{mc}")
                        nc.vector.tensor_copy(ptT[:, :384], ptps[mc][:, :384])
                        nc.scalar.copy(ptT[:, 384:], ptps[mc][:, 384:])
                        ptts[mc] = ptT
                    for hc in range(2):
                        nc.tensor.matmul(
                            po[hc],
                            v_sb[mc][:, hc * P : (hc + 1) * P],
                            ptts[mc][:, nh * 512 : (nh + 1) * 512],
                            start=(mc == 0),
                            stop=(mc == NSUB - 1),
                        )
                for hc in range(2):
                    nc.scalar.copy(ot_sb[hc][:, nh * 512 : (nh + 1) * 512],
                                   po[hc][:])
            st[b]["ot"] = ot_sb

        def y_phase(b):
            ot_sb, d_pack = st[b]["ot"], st[b]["dp"]
            # ---- Y = invd * relu(O^T.T @ Wo + d*bo) ----
            invd_pack = sb.tile([P, NSUB], f32, tag="ivp", bufs=2,
                                name=f"ivp{b}")
            nc.vector.reciprocal(invd_pack[:], d_pack[:])
            invd_tiles = [invd_pack[:, ns : ns + 1] for ns in range(NSUB)]
            if not zero_bias:
                pdr = ps.tile([NSUB, P], f32, tag="s", bufs=3, name=f"pdr{b}")
                nc.tensor.transpose(pdr[:], d_pack[:], id_f32[:])
                drow_pack = sb.tile([NSUB, P], bf16, tag="drow", bufs=2,
                                    name=f"drow{b}")
                nc.vector.tensor_copy(drow_pack[:], pdr[:])
                drow_flat = sb.tile([1, N], bf16, tag="drowf", bufs=2,
                                    name=f"drowf{b}")
                for ns in range(NSUB):
                    nc.gpsimd.dma_start(
                        drow_flat[:, ns * P : (ns + 1) * P],
                        drow_pack[ns : ns + 1, :])
                drow_tiles = [drow_flat[:, ns * P : (ns + 1) * P]
                              for ns in range(NSUB)]
            for ns in range(NSUB):
                nsl = slice(ns * P, (ns + 1) * P)
                py = ps.tile([P, DOUT], f32, tag="s", bufs=3, name=f"py{b}_{ns}")
                for hc in range(2):
                    nc.tensor.matmul(
                        py[:], ot_sb[hc][:, nsl], wo_sb[hc][:],
                        start=(hc == 0), stop=(zero_bias and hc == 1),
                    )
                if not zero_bias:
                    nc.tensor.matmul(py[:], drow_tiles[ns][:], bo_sb[:],
                                     start=False, stop=True)
                y = sb.tile([P, DOUT], f32, tag="y", bufs=8, name=f"y{b}_{ns}")
                nc.vector.tensor_scalar(
                    out=y[:], in0=py[:], scalar1=invd_tiles[ns][:],
                    scalar2=0.0, op0=ALU.mult, op1=ALU.max,
                )
                nc.sync.dma_start(out_d[b, nsl, :], y[:])
            del st[b]

        # phase-interleaved emission: keep PE fed with the next batch's
        # matmuls while ACT/DVE work through the current batch's epilogues
        qkv_phase(0)
        s_phase(0)
        for b in range(bp):
            if b + 1 < bp:
                qkv_phase(b + 1)
            trav_phase(b)
            if b + 1 < bp:
                s_phase(b + 1)
            y_phase(b)

    nc.compile()
    return nc


def _get_nc(bp=BP, zero_bias=False):
    key = (bp, zero_bias)
    if key not in _nc_cache:
        _nc_cache[key] = _build_nc(bp, zero_bias)
    return _nc_cache[key]


def kernel(x, mask, Wv, bv, Wk, bk, Wq, bq, Wo, bo):
    global last_results
    import ml_dtypes
    from concourse.bass_utils import run_bass_kernel_spmd

    bf = ml_dtypes.bfloat16
    x = np.asarray(x, np.float32)
    xT = np.ascontiguousarray(x.transpose(0, 2, 1)).astype(bf)  # [B, D, N]
    mk = ((np.asarray(mask, np.float32) - 1.0) * 9.0e15).astype(bf)
    w = {
        "Wq": np.ascontiguousarray(np.asarray(Wq, np.float32)).astype(bf),
        "Wk": np.ascontiguousarray(np.asarray(Wk, np.float32)).astype(bf),
        "Wv": np.ascontiguousarray(np.asarray(Wv, np.float32)).astype(bf),
        "Wo": np.ascontiguousarray(np.asarray(Wo, np.float32)).astype(bf),
        "bq": np.asarray(bq, np.float32).reshape(H, 1).copy(),
        "bk": np.asarray(bk, np.float32).reshape(H, 1).copy(),
        "bv": np.asarray(bv, np.float32).reshape(1, H).astype(bf),
        "bo": np.asarray(bo, np.float32).reshape(1, DOUT).astype(bf),
        "ones": np.ones((1, P), bf),
    }

    zero_bias = not (np.any(np.asarray(w["bv"], np.float32))
                     or np.any(np.asarray(w["bo"], np.float32)))
    nc = _get_nc(BP, zero_bias)
    in_maps = []
    for c in range(NCORES):
        sl = slice(c * BP, (c + 1) * BP)
        m = {"xT": np.ascontiguousarray(xT[sl]),
             "mask": np.ascontiguousarray(mk[sl])}
        m.update(w)
        in_maps.append(m)

    trace = bool(int(os.environ.get("BASS_KERNEL_TRACE", "0")))
    try:
        res = run_bass_kernel_spmd(
            nc, in_maps, core_ids=list(range(NCORES)), trace=trace
        )
    except Exception:
        if not trace:
            raise
        res = run_bass_kernel_spmd(nc, in_maps, core_ids=list(range(NCORES)))
    last_results = res
    out = np.concatenate([r["out"] for r in res.results], axis=0)
    return np.ascontiguousarray(out.astype(np.float32))


if __name__ == "__main__":
    nc = _get_nc(1)
    print("built ok:", nc)



# revision 3
# speedup vs baseline: 1.6044x; 1.6044x over previous
"""Trainium2 Bass kernel for nn_AttModel (masked GNN attention).

Reference computation (per batch b of 32, N=1024, D=H=O=256):
    v = relu(x @ Wv); q = relu(x @ Wq); k = relu(x @ Wk)      (biases zero)
    S = q @ k^T
    att = softmax(S * mask - 9e15 * (1 - mask), axis=-1)
    out = relu((att @ v) @ Wo)

Strategy: pure data parallelism over batch — 8 NeuronCores, 4 batches
each, weights replicated, no collectives.  Per batch, computed in the
TRANSPOSED score orientation S^T[m, n] so the post-softmax matrix feeds
the AV matmul directly (no PE transposes at all):

  - Q^T, K^T [h, n] = relu(W^T x^T) in bf16 matmuls, epilogues write
    fp8(e4m3) packs [p, hc, n]; V [m, h] epilogue writes fp8 packs
    [p, i, h] pairing m-chunks for DoubleRow.
  - Mask handling costs no vector-engine work: the host sends
    maskT-1 in {-1, 0} as fp8; the PE injects M = 240*(maskT-1) into
    PSUM via a DoubleRow matmul with stationary [240*I, 0] (or
    [0, 240*I] for the other n-half), then the S^T = K^T^T Q^T fp8
    DoubleRow matmuls accumulate on top.  Masked scores sit at ~-236,
    so exp underflows to exactly 0.
  - One ACT pass per m-pair does exp(S' - 6) straight from PSUM into
    fp8 P~^T tiles (the -6 shift keeps exp under fp8 e4m3 max 240 and
    cancels in the softmax normalization).
  - Row sums d (softmax denominators) come out as PSUM columns from
    F=1 DoubleRow matmuls with a ones vector: d[n-chunk] =
    (P~^T chunk)^T @ 1; reciprocal on DVE.
  - O^T = V^T P~^T via fp8 DoubleRow (K=256 per step), Y = O^T^T Wo in
    bf16, final epilogue fuses the deferred softmax normalization:
    y = max(py * (1/d), 0).

fp8 is applied only where measured error allows (q,k,P~,v): end-to-end
fro error ~1e-2 vs the fp32 reference (budget 2e-2); projections and
the output matmul stay bf16.
"""

import os

import numpy as np

B, N, DIN, H, DOUT = 32, 1024, 256, 256, 256
NCORES = 8
BP = B // NCORES  # batches per core
P = 128
NSUB = N // P     # 8 row-chunks of 128
NHALF = N // 512  # 2 column-halves of 512
EXP_SHIFT = -6.0  # exp(S - 6): keeps exp(S) under fp8 max; cancels in softmax

_nc_cache = {}
last_results = None  # BassKernelResults of the most recent run (for test.py)


def _build_nc(bp=BP, zero_bias=True):
    import concourse.mybir as mybir
    import concourse.tile as tile
    from concourse import bacc
    from contextlib import ExitStack

    f32 = mybir.dt.float32
    bf16 = mybir.dt.bfloat16
    fp8 = mybir.dt.float8e4
    AF = mybir.ActivationFunctionType
    ALU = mybir.AluOpType
    DR = mybir.MatmulPerfMode.DoubleRow

    nc = bacc.Bacc("TRN2", target_bir_lowering=False)

    xT_d = nc.declare_dram_parameter("xT", [bp, DIN, N], bf16, isOutput=False)
    mask_d = nc.declare_dram_parameter("maskT8", [bp, N, N], fp8, isOutput=False)
    wq_d = nc.declare_dram_parameter("Wq", [DIN, H], bf16, isOutput=False)
    wk_d = nc.declare_dram_parameter("Wk", [DIN, H], bf16, isOutput=False)
    wv_d = nc.declare_dram_parameter("Wv", [DIN, H], bf16, isOutput=False)
    wo_d = nc.declare_dram_parameter("Wo", [H, DOUT], bf16, isOutput=False)
    bq_d = nc.declare_dram_parameter("bq", [H, 1], f32, isOutput=False)
    bk_d = nc.declare_dram_parameter("bk", [H, 1], f32, isOutput=False)
    bv_d = nc.declare_dram_parameter("bv", [1, H], bf16, isOutput=False)
    bo_d = nc.declare_dram_parameter("bo", [1, DOUT], bf16, isOutput=False)
    i240_d = nc.declare_dram_parameter("i240", [P, 512], fp8, isOutput=False)
    ones8_d = nc.declare_dram_parameter("ones8", [P, 2], fp8, isOutput=False)
    ones_row_d = nc.declare_dram_parameter("ones_row", [1, N], bf16,
                                           isOutput=False)
    out_d = nc.declare_dram_parameter("out", [bp, N, DOUT], f32, isOutput=True)

    with tile.TileContext(nc) as tc, ExitStack() as ctx:
        const = ctx.enter_context(tc.tile_pool(name="const", bufs=1))
        sb = ctx.enter_context(tc.tile_pool(name="sb", bufs=1))
        ps = ctx.enter_context(tc.tile_pool(name="ps", bufs=1, space="PSUM"))

        # ---- constants / weights (loaded once) ----
        # w*_pack[p, c*H + h] = W[c*P + p, h]
        def load_w(d, nm, width):
            t = const.tile([P, 2 * width], bf16, tag=nm, name=nm)
            nc.sync.dma_start(
                t[:].rearrange("p (c h) -> p c h", c=2),
                d.rearrange("(c p) h -> p c h", c=2))
            return t

        wq_sb = load_w(wq_d, "wqp", H)
        wk_sb = wv_sb = wo_sb = None  # loaded after batch-0 xt DMAs

        i240 = const.tile([P, 512], fp8, tag="i240", name="i240")
        nc.sync.dma_start(i240[:], i240_d[:, :])
        ones8 = const.tile([P, 2], fp8, tag="ones8", name="ones8")
        nc.gpsimd.dma_start(ones8[:], ones8_d[:, :])
        bias_sh = const.tile([P, 1], f32, tag="bsh", name="bias_sh")
        nc.vector.memset(bias_sh[:], EXP_SHIFT)

        if not zero_bias:
            def load_b(d, nm):
                t = const.tile([P, 2], f32, tag=nm, name=nm)
                nc.gpsimd.dma_start(
                    t[:].rearrange("p (c o) -> p c o", c=2),
                    d.rearrange("(c p) o -> p c o", c=2))
                return [t[:, c: c + 1] for c in range(2)]

            bq_sb = load_b(bq_d, "bqp")
            bk_sb = load_b(bk_d, "bkp")
            bv_sb = const.tile([1, H], bf16, tag="bv", name="bv_sb")
            nc.gpsimd.dma_start(bv_sb[:], bv_d[:, :])
            bo_sb = const.tile([1, DOUT], bf16, tag="bo", name="bo_sb")
            nc.gpsimd.dma_start(bo_sb[:], bo_d[:, :])
            ones_row = const.tile([1, N], bf16, tag="onesr", name="ones_row")
            nc.gpsimd.dma_start(ones_row[:], ones_row_d[:, :])
        else:
            bq_sb = bk_sb = bv_sb = bo_sb = ones_row = None

        # shifted exp bias per h-chunk for q/k epilogues (general path)
        st = {}

        def qkv_phase(b):
            nonlocal wk_sb, wv_sb, wo_sb
            xt = []
            for c in range(2):
                t = sb.tile([P, N], bf16, tag=f"xt{c}", bufs=3, name=f"xt{c}_{b}")
                nc.sync.dma_start(t[:], xT_d[b, c * P: (c + 1) * P, :])
                xt.append(t)
            if wk_sb is None:
                wk_sb = load_w(wk_d, "wkp", H)
                wv_sb = load_w(wv_d, "wvp", H)
                wo_sb = load_w(wo_d, "wop", DOUT)

            # Q^T, K^T -> fp8 packs [p, hc*N + n]
            qt8 = sb.tile([P, 2 * N], fp8, tag="qt8", bufs=2, name=f"qt8_{b}")
            kt8 = sb.tile([P, 2 * N], fp8, tag="kt8", bufs=2, name=f"kt8_{b}")
            for hc in range(2):
                for nh in range(NHALF):
                    nsl = slice(nh * 512, (nh + 1) * 512)
                    osl = slice(hc * N + nh * 512, hc * N + nh * 512 + 512)
                    pq = ps.tile([P, 512], f32, tag="s", bufs=2,
                                 name=f"pq{b}_{hc}_{nh}")
                    for dc in range(2):
                        nc.tensor.matmul(
                            pq[:],
                            wq_sb[:, dc * H + hc * P: dc * H + (hc + 1) * P],
                            xt[dc][:, nsl],
                            start=(dc == 0), stop=(dc == 1),
                        )
                    if zero_bias:
                        nc.scalar.activation(qt8[:, osl], pq[:], AF.Relu)
                    else:
                        nc.scalar.activation(qt8[:, osl], pq[:], AF.Relu,
                                             bias=bq_sb[hc][:])
                    pk = ps.tile([P, 512], f32, tag="s", bufs=2,
                                 name=f"pk{b}_{hc}_{nh}")
                    for dc in range(2):
                        nc.tensor.matmul(
                            pk[:],
                            wk_sb[:, dc * H + hc * P: dc * H + (hc + 1) * P],
                            xt[dc][:, nsl],
                            start=(dc == 0), stop=(dc == 1),
                        )
                    if zero_bias:
                        nc.vector.tensor_scalar_max(kt8[:, osl], pk[:], 0.0)
                    else:
                        nc.vector.tensor_scalar(
                            out=kt8[:, osl], in0=pk[:], scalar1=bk_sb[hc][:],
                            scalar2=0.0, op0=ALU.add, op1=ALU.max,
                        )

            # V -> fp8 pair-packs v8[a][p, i*H + h], m = a*256 + i*128 + p
            v8 = []
            for a in range(4):
                t = sb.tile([P, 2 * H], fp8, tag=f"v8_{a}", bufs=2,
                            name=f"v8_{a}_{b}")
                v8.append(t)
            for mc in range(NSUB):
                msl = slice(mc * P, (mc + 1) * P)
                pv = ps.tile([P, H], f32, tag="sv", bufs=2, name=f"pv{b}_{mc}")
                for dc in range(2):
                    nc.tensor.matmul(
                        pv[:], xt[dc][:, msl],
                        wv_sb[:, dc * H: (dc + 1) * H],
                        start=(dc == 0), stop=(zero_bias and dc == 1),
                    )
                if not zero_bias:
                    nc.tensor.matmul(pv[:], ones_row[:, 0:P], bv_sb[:],
                                     start=False, stop=True)
                a, i = mc // 2, mc % 2
                nc.vector.tensor_scalar_max(
                    v8[a][:, i * H: (i + 1) * H], pv[:], 0.0)
            st[b] = {"qt8": qt8, "kt8": kt8, "v8": v8}

        def s_phase(b):
            qt8, kt8 = st[b]["qt8"], st[b]["kt8"]
            qt3 = qt8[:].rearrange("p (i n) -> p i n", i=2)
            kt3 = kt8[:].rearrange("p (i n) -> p i n", i=2)
            i240a = i240[:, 0:256].rearrange("p (i m) -> p i m", i=2)
            i240b = i240[:, 256:512].rearrange("p (i m) -> p i m", i=2)
            pmt = []
            for a in range(4):
                t = sb.tile([P, 2 * N], fp8, tag=f"pmt{a}", bufs=2,
                            name=f"pmt{a}_{b}")
                pmt.append(t)
            for mc in range(NSUB):
                mk = sb.tile([P, N], fp8, tag="mask", bufs=10,
                             name=f"mk{b}_{mc}")
                nc.sync.dma_start(mk[:], mask_d[b, mc * P: (mc + 1) * P, :])
                mk3 = mk[:].rearrange("p (i n) -> p i n", i=2)

                ss = ps.tile([P, N], f32, tag="ss", bufs=2, name=f"ss{b}_{mc}")
                for nh in range(NHALF):
                    osl = slice(nh * 512, (nh + 1) * 512)
                    inj = i240a if nh == 0 else i240b
                    nc.tensor.matmul(ss[:, osl], inj, mk3,
                                     start=True, stop=False, perf_mode=DR)
                    nc.tensor.matmul(
                        ss[:, osl], kt3[:, :, mc * P: (mc + 1) * P],
                        qt3[:, :, osl],
                        start=False, stop=True, perf_mode=DR)
                a, i = mc // 2, mc % 2
                nc.scalar.activation(pmt[a][:, i * N: (i + 1) * N], ss[:],
                                     AF.Exp, bias=bias_sh[:])
            st[b]["pmt"] = pmt

        def dred_phase(b):
            pmt = st[b]["pmt"]
            ones3 = ones8[:].rearrange("p (i o) -> p i o", i=2)
            pd = ps.tile([P, NSUB], f32, tag="sv", bufs=2, name=f"pd{b}")
            for nct in range(NSUB):
                ncsl = slice(nct * P, (nct + 1) * P)
                for a in range(4):
                    p3 = pmt[a][:].rearrange("p (i n) -> p i n", i=2)
                    nc.tensor.matmul(
                        pd[:, nct: nct + 1], p3[:, :, ncsl], ones3,
                        start=(a == 0), stop=(a == 3), perf_mode=DR,
                        skip_group_check=True)
            invd = sb.tile([P, NSUB], f32, tag="ivd", bufs=2, name=f"ivd{b}")
            nc.vector.reciprocal(invd[:], pd[:])
            st[b]["invd"] = invd

        def av_phase(b):
            pmt, v8 = st[b]["pmt"], st[b]["v8"]
            ot = [
                sb.tile([P, N], bf16, tag=f"ot{hc}", bufs=2, name=f"ot{hc}_{b}")
                for hc in range(2)
            ]
            for nh in range(NHALF):
                nsl = slice(nh * 512, (nh + 1) * 512)
                for hc in range(2):
                    po = ps.tile([P, 512], f32, tag="s", bufs=2,
                                 name=f"po{b}_{hc}_{nh}")
                    for a in range(4):
                        p3 = pmt[a][:].rearrange("p (i n) -> p i n", i=2)
                        v3 = v8[a][:].rearrange("p (i h) -> p i h", i=2)
                        nc.tensor.matmul(
                            po[:], v3[:, :, hc * P: (hc + 1) * P],
                            p3[:, :, nsl],
                            start=(a == 0), stop=(a == 3), perf_mode=DR)
                    if hc == 0:
                        nc.scalar.copy(ot[hc][:, nsl], po[:])
                    else:
                        nc.vector.tensor_copy(ot[hc][:, nsl], po[:])
            st[b]["ot"] = ot

        def y_phase(b):
            ot, invd = st[b]["ot"], st[b]["invd"]
            if not zero_bias:
                # d row for the bo bias: transpose invd's source d... general
                # path: recompute d = 1/invd is wasteful; instead pack d rows
                # via gpsimd DMA from a DVE copy of pd. Keep it simple: the
                # harness always has zero biases; general path adds d*bo via
                # K=1 matmuls from a flattened d-row.
                pdr = ps.tile([NSUB, P], f32, tag="sdr", bufs=2, name=f"pdr{b}")
                dcol = sb.tile([P, NSUB], f32, tag="dcol", bufs=2,
                               name=f"dcol{b}")
                nc.vector.reciprocal(dcol[:], invd[:])  # back to d
                id128 = st.setdefault("_id128", None)
                if id128 is None:
                    from concourse.masks import make_identity
                    id128 = const.tile([P, P], f32, tag="idf32", name="id_f32")
                    make_identity(nc, id128[:])
                    st["_id128"] = id128
                nc.tensor.transpose(pdr[:], dcol[:], id128[:])
                drow_pack = sb.tile([NSUB, P], bf16, tag="drow", bufs=2,
                                    name=f"drow{b}")
                nc.vector.tensor_copy(drow_pack[:], pdr[:])
                drow_flat = sb.tile([1, N], bf16, tag="drowf", bufs=2,
                                    name=f"drowf{b}")
                for nct in range(NSUB):
                    nc.gpsimd.dma_start(
                        drow_flat[:, nct * P: (nct + 1) * P],
                        drow_pack[nct: nct + 1, :])
            for nct in range(NSUB):
                ncsl = slice(nct * P, (nct + 1) * P)
                py = ps.tile([P, DOUT], f32, tag="sv", bufs=2,
                             name=f"py{b}_{nct}")
                for hc in range(2):
                    nc.tensor.matmul(
                        py[:], ot[hc][:, ncsl],
                        wo_sb[:, hc * DOUT: (hc + 1) * DOUT],
                        start=(hc == 0), stop=(zero_bias and hc == 1),
                    )
                if not zero_bias:
                    nc.tensor.matmul(
                        py[:], drow_flat[:, ncsl], bo_sb[:],
                        start=False, stop=True)
                y = sb.tile([P, DOUT], f32, tag="y", bufs=6, name=f"y{b}_{nct}")
                nc.vector.tensor_scalar(
                    out=y[:], in0=py[:], scalar1=invd[:, nct: nct + 1],
                    scalar2=0.0, op0=ALU.mult, op1=ALU.max,
                )
                nc.sync.dma_start(out_d[b, ncsl, :], y[:])
            del st[b]

        # phase-interleaved emission: keep the PE fed with the next batch's
        # matmuls while ACT/DVE work through the current batch's epilogues
        qkv_phase(0)
        s_phase(0)
        for b in range(bp):
            if b + 1 < bp:
                qkv_phase(b + 1)
            dred_phase(b)
            av_phase(b)
            if b + 1 < bp:
                s_phase(b + 1)
            y_phase(b)

    nc.compile()
    return nc


def _get_nc(bp=BP, zero_bias=True):
    key = (bp, zero_bias)
    if key not in _nc_cache:
        _nc_cache[key] = _build_nc(bp, zero_bias)
    return _nc_cache[key]


def kernel(x, mask, Wv, bv, Wk, bk, Wq, bq, Wo, bo):
    global last_results
    import ml_dtypes
    from concourse.bass_utils import run_bass_kernel_spmd

    bf = ml_dtypes.bfloat16
    e4 = ml_dtypes.float8_e4m3
    x = np.asarray(x, np.float32)
    xT = np.ascontiguousarray(x.transpose(0, 2, 1)).astype(bf)  # [B, D, N]
    # maskT - 1 in {-1, 0}, fp8; PE injects *240 into the score PSUM
    mkT8 = (np.asarray(mask, np.float32).transpose(0, 2, 1) - 1.0).astype(e4)
    mkT8 = np.ascontiguousarray(mkT8)

    i240 = np.zeros((P, 512), e4)
    idx = np.arange(P)
    i240[idx, idx] = 240.0          # [240*I, 0]  (n-half 0)
    i240[idx, 384 + idx] = 240.0    # [0, 240*I]  (n-half 1)

    w = {
        "Wq": np.ascontiguousarray(np.asarray(Wq, np.float32)).astype(bf),
        "Wk": np.ascontiguousarray(np.asarray(Wk, np.float32)).astype(bf),
        "Wv": np.ascontiguousarray(np.asarray(Wv, np.float32)).astype(bf),
        "Wo": np.ascontiguousarray(np.asarray(Wo, np.float32)).astype(bf),
        "bq": np.asarray(bq, np.float32).reshape(H, 1).copy(),
        "bk": np.asarray(bk, np.float32).reshape(H, 1).copy(),
        "bv": np.asarray(bv, np.float32).reshape(1, H).astype(bf),
        "bo": np.asarray(bo, np.float32).reshape(1, DOUT).astype(bf),
        "i240": i240,
        "ones8": np.ones((P, 2), e4),
        "ones_row": np.ones((1, N), bf),
    }

    zero_bias = not (np.any(np.asarray(bq, np.float32))
                     or np.any(np.asarray(bk, np.float32))
                     or np.any(np.asarray(bv, np.float32))
                     or np.any(np.asarray(bo, np.float32)))
    nc = _get_nc(BP, zero_bias)
    in_maps = []
    for c in range(NCORES):
        sl = slice(c * BP, (c + 1) * BP)
        m = {"xT": np.ascontiguousarray(xT[sl]),
             "maskT8": np.ascontiguousarray(mkT8[sl])}
        m.update(w)
        in_maps.append(m)

    trace = bool(int(os.environ.get("BASS_KERNEL_TRACE", "0")))
    try:
        res = run_bass_kernel_spmd(
            nc, in_maps, core_ids=list(range(NCORES)), trace=trace
        )
    except Exception:
        if not trace:
            raise
        res = run_bass_kernel_spmd(nc, in_maps, core_ids=list(range(NCORES)))
    last_results = res
    out = np.concatenate([r["out"] for r in res.results], axis=0)
    return np.ascontiguousarray(out.astype(np.float32))


if __name__ == "__main__":
    nc = _get_nc(1)
    print("built ok:", nc)


# revision 56
# speedup vs baseline: 1.9796x; 1.2339x over previous
"""Trainium2 Bass kernel for nn_AttModel (masked GNN attention).

Reference computation (per batch b of 32, N=1024, D=H=O=256):
    v = relu(x @ Wv); q = relu(x @ Wq); k = relu(x @ Wk)      (biases zero)
    S = q @ k^T
    att = softmax(S * mask - 9e15 * (1 - mask), axis=-1)
    out = relu((att @ v) @ Wo)

Strategy: pure data parallelism over batch — 8 NeuronCores, 4 batches
each, weights replicated, no collectives.  Per batch, computed in the
TRANSPOSED score orientation S^T[m, n] so the post-softmax matrix feeds
the AV matmul directly (no PE transposes at all):

  - Q^T, K^T [h, n] = relu(W^T x^T) in bf16 matmuls, epilogues write
    fp8(e4m3) packs [p, hc, n]; V [m, h] epilogue writes fp8 packs
    [p, i, h] pairing m-chunks for DoubleRow.
  - Mask handling costs no vector-engine work: the host sends
    maskT-1 in {-1, 0} as fp8; the PE injects M = 240*(maskT-1) into
    PSUM via a DoubleRow matmul with stationary [240*I, 0] (or
    [0, 240*I] for the other n-half), then the S^T = K^T^T Q^T fp8
    DoubleRow matmuls accumulate on top.  Masked scores sit at ~-236,
    so exp underflows to exactly 0.
  - One ACT pass per m-pair does exp(S' - 6) straight from PSUM into
    fp8 P~^T tiles (the -6 shift keeps exp under fp8 e4m3 max 240 and
    cancels in the softmax normalization).
  - Row sums d (softmax denominators) come out as PSUM columns from
    F=1 DoubleRow matmuls with a ones vector: d[n-chunk] =
    (P~^T chunk)^T @ 1; reciprocal on DVE.
  - O^T = V^T P~^T via fp8 DoubleRow (K=256 per step), Y = O^T^T Wo in
    bf16, final epilogue fuses the deferred softmax normalization:
    y = max(py * (1/d), 0).

fp8(e4m3) is applied only where measured error allows: q, k, P~, v
values plus the K- and V-projection matmuls (DoubleRow, fp8 x and W);
the Q-projection and the output matmul stay bf16.  End-to-end error vs
the fp32 reference: fro 0.0149, absmax/scale 0.0169 (budget 2e-2).

Schedule: software-pipelined across the 4 batches (prefetch 2 ahead;
qkv(b+1) emitted mid-s(b) so its epilogues interleave between exps on
ACT/DVE; av/y of batch b-1 emitted after s(b) so next-batch S matmuls
outrank them on the PE).  PSUM tag layout (8 banks): ss 2x[128,1024]
(scores), s 2x[128,512] (q/k psums + d columns), sv 2x[128,512]
(V/AV/Y psums).  A PE warmup chain covers the initial DMA wait so the
p-state ramp finishes before real matmuls.  The first/last batches
run their score matmuls + exps in nh-major order with half-size score
tiles, shortening pipeline fill and drain; batch 0's x inputs arrive
in staged n-half DMAs and the fp8 constants (Wk8 pack, 240*I pair,
ones) ship as one host-prepacked DMA to cut head dispatch slots.
Cost-model (TimelineSim) exec estimate: 61.9 us/core (baseline bf16
kernel: 122.5 us).
"""

import os

import numpy as np

B, N, DIN, H, DOUT = 32, 1024, 256, 256, 256
NCORES = 8
BP = B // NCORES  # batches per core
P = 128
NSUB = N // P     # 8 row-chunks of 128
NHALF = N // 512  # 2 column-halves of 512
EXP_SHIFT = -6.0  # exp(S - 6): keeps exp(S) under fp8 max; cancels in softmax

_nc_cache = {}
last_results = None  # BassKernelResults of the most recent run (for test.py)


def _build_nc(bp=BP, zero_bias=True):
    import concourse.mybir as mybir
    import concourse.tile as tile
    from concourse import bacc
    from contextlib import ExitStack

    f32 = mybir.dt.float32
    bf16 = mybir.dt.bfloat16
    fp8 = mybir.dt.float8e4
    AF = mybir.ActivationFunctionType
    ALU = mybir.AluOpType
    DR = mybir.MatmulPerfMode.DoubleRow

    nc = bacc.Bacc("TRN2", target_bir_lowering=False)

    xT_d = nc.declare_dram_parameter("xT", [bp, DIN, N], bf16, isOutput=False)
    xT8_d = nc.declare_dram_parameter("xT8", [bp, DIN, N], fp8, isOutput=False)
    kc_d = nc.declare_dram_parameter("kconst", [P, 1026], fp8, isOutput=False)
    wv8_d = nc.declare_dram_parameter("Wv8", [DIN, H], fp8, isOutput=False)
    mask_d = nc.declare_dram_parameter("maskT8", [bp, N, N], fp8, isOutput=False)
    wq_d = nc.declare_dram_parameter("Wq", [DIN, H], bf16, isOutput=False)
    wk_d = nc.declare_dram_parameter("Wk", [DIN, H], bf16, isOutput=False)
    wv_d = nc.declare_dram_parameter("Wv", [DIN, H], bf16, isOutput=False)
    wo_d = nc.declare_dram_parameter("Wo", [H, DOUT], bf16, isOutput=False)
    bq_d = nc.declare_dram_parameter("bq", [H, 1], f32, isOutput=False)
    bk_d = nc.declare_dram_parameter("bk", [H, 1], f32, isOutput=False)
    bv_d = nc.declare_dram_parameter("bv", [1, H], bf16, isOutput=False)
    bo_d = nc.declare_dram_parameter("bo", [1, DOUT], bf16, isOutput=False)
    ones_row_d = nc.declare_dram_parameter("ones_row", [1, N], bf16,
                                           isOutput=False)
    out_d = nc.declare_dram_parameter("out", [bp, N, DOUT], f32, isOutput=True)

    with tile.TileContext(nc) as tc, ExitStack() as ctx:
        const = ctx.enter_context(tc.tile_pool(name="const", bufs=1))
        sb = ctx.enter_context(tc.tile_pool(name="sb", bufs=1))
        ps = ctx.enter_context(tc.tile_pool(name="ps", bufs=1, space="PSUM"))

        # ---- constants / weights (loaded once) ----
        # w*_pack[p, c*H + h] = W[c*P + p, h]
        def load_w(d, nm, width, dt=bf16):
            t = const.tile([P, 2 * width], dt, tag=nm, name=nm)
            nc.sync.dma_start(
                t[:].rearrange("p (c h) -> p c h", c=2),
                d.rearrange("(c p) h -> p c h", c=2))
            return t

        wq_sb = wk_sb = wv_sb = wo_sb = None  # loaded after batch-0 xt DMA
        # one host-prepacked fp8 const DMA: [wk8 pack | 240*I pair | ones]
        kconst = const.tile([P, 1026], fp8, tag="kc", name="kconst")
        i240 = kconst[:, 512:1024]
        ones8 = kconst[:, 1024:1026]
        bias_sh = const.tile([P, 1], f32, tag="bsh", name="bias_sh")

        if not zero_bias:
            def load_b(d, nm):
                t = const.tile([P, 2], f32, tag=nm, name=nm)
                nc.gpsimd.dma_start(
                    t[:].rearrange("p (c o) -> p c o", c=2),
                    d.rearrange("(c p) o -> p c o", c=2))
                return [t[:, c: c + 1] for c in range(2)]

            bq_sb = load_b(bq_d, "bqp")
            bk_sb = load_b(bk_d, "bkp")
            bv_sb = const.tile([1, H], bf16, tag="bv", name="bv_sb")
            nc.gpsimd.dma_start(bv_sb[:], bv_d[:, :])
            bo_sb = const.tile([1, DOUT], bf16, tag="bo", name="bo_sb")
            nc.gpsimd.dma_start(bo_sb[:], bo_d[:, :])
            ones_row = const.tile([1, N], bf16, tag="onesr", name="ones_row")
            nc.gpsimd.dma_start(ones_row[:], ones_row_d[:, :])
        else:
            bq_sb = bk_sb = bv_sb = bo_sb = ones_row = None

        # shifted exp bias per h-chunk for q/k epilogues (general path)
        st = {}

        def prefetch_phase(b):
            nonlocal wq_sb, wk_sb, wv_sb, wo_sb
            # one DMA for both d-chunks of x^T: xt_t[p, c*N + n] = xT[c*P+p, n]
            xt_t = sb.tile([P, 2 * N], bf16, tag="xt", bufs=4, name=f"xt_{b}")
            xt8_t = sb.tile([P, 2 * N], fp8, tag="xt8", bufs=4, name=f"xt8_{b}")
            x3 = xt_t[:].rearrange("p (c n) -> p c n", c=2)
            x3d = xT_d[b].rearrange("(c p) n -> p c n", c=2)
            x83 = xt8_t[:].rearrange("p (c n) -> p c n", c=2)
            x83d = xT8_d[b].rearrange("(c p) n -> p c n", c=2)
            mkg = [sb.tile([P, 4 * N], fp8, tag="mask", bufs=6,
                           name=f"mk{b}_{g}") for g in range(2)]

            def mask_dma(g):
                nc.sync.dma_start(
                    mkg[g][:].rearrange("p (c n) -> p c n", c=4),
                    mask_d[b, g * 512: (g + 1) * 512, :]
                    .rearrange("(c p) n -> p c n", c=4))

            if b == 0:
                # batch 0 is nh-major: phase A only touches the first
                # n-halves of x/x8, so stage the input DMAs accordingly
                nc.sync.dma_start(x3[:, :, 0:512], x3d[:, :, 0:512])
                nc.sync.dma_start(x83[:, :, 0:512], x83d[:, :, 0:512])
                wq_sb = load_w(wq_d, "wqp", H)
                nc.sync.dma_start(kconst[:], kc_d[:, :])
                wk_sb = kconst[:, 0:512]
                nc.vector.memset(bias_sh[:], EXP_SHIFT)
                mask_dma(0)
                nc.sync.dma_start(x3[:, :, 512:1024], x3d[:, :, 512:1024])
                nc.sync.dma_start(x83[:, :, 512:1024], x83d[:, :, 512:1024])
                mask_dma(1)
            else:
                nc.sync.dma_start(x3, x3d)
                nc.sync.dma_start(x83, x83d)
                mask_dma(0)
                mask_dma(1)
            st.setdefault("xt", {})[b] = xt_t
            st.setdefault("xt8", {})[b] = xt8_t
            st.setdefault("mkg", {})[b] = mkg
            if wv_sb is None:
                # after the masks: V/Y weights aren't needed until later
                wv_sb = load_w(wv8_d, "wvp", H, fp8)
                wo_sb = load_w(wo_d, "wop", DOUT)

        def qkv_phase(b):
            xt_t = st["xt"].pop(b)
            xt = [xt_t[:, c * N: (c + 1) * N] for c in range(2)]
            xt8_t = st["xt8"].pop(b)
            x83 = xt8_t[:].rearrange("p (c n) -> p c n", c=2)
            wk3 = wk_sb.rearrange("p (c h) -> p c h", c=2)
            wv3 = wv_sb[:].rearrange("p (c h) -> p c h", c=2)

            # Q^T, K^T -> fp8 packs [p, hc*N + n].  nh-outer so both h-chunks
            # of an n-half finish first; epilogues alternate ACT/DVE so the
            # two chunks the next exp needs land in parallel.
            qt8 = sb.tile([P, 2 * N], fp8, tag="qt8", bufs=3, name=f"qt8_{b}")
            kt8 = sb.tile([P, 2 * N], fp8, tag="kt8", bufs=3, name=f"kt8_{b}")
            for nh in range(NHALF):
                for hc in range(2):
                    nsl = slice(nh * 512, (nh + 1) * 512)
                    osl = slice(hc * N + nh * 512, hc * N + nh * 512 + 512)
                    pq = ps.tile([P, 512], f32, tag="s", bufs=2,
                                 name=f"pq{b}_{hc}_{nh}")
                    for dc in range(2):
                        nc.tensor.matmul(
                            pq[:],
                            wq_sb[:, dc * H + hc * P: dc * H + (hc + 1) * P],
                            xt[dc][:, nsl],
                            start=(dc == 0), stop=(dc == 1),
                        )
                    qbias = None if zero_bias else bq_sb[hc][:]
                    if hc == 0:
                        if qbias is None:
                            nc.scalar.activation(qt8[:, osl], pq[:], AF.Relu)
                        else:
                            nc.scalar.activation(qt8[:, osl], pq[:], AF.Relu,
                                                 bias=qbias)
                    else:
                        nc.vector.tensor_scalar(
                            out=qt8[:, osl], in0=pq[:],
                            scalar1=0.0 if zero_bias else qbias,
                            scalar2=0.0, op0=ALU.add, op1=ALU.max,
                        )
                    pk = ps.tile([P, 512], f32, tag="s", bufs=2,
                                 name=f"pk{b}_{hc}_{nh}")
                    nc.tensor.matmul(
                        pk[:], wk3[:, :, hc * P: (hc + 1) * P],
                        x83[:, :, nsl], start=True, stop=True, perf_mode=DR)
                    kbias = None if zero_bias else bk_sb[hc][:]
                    if hc == 1 and nh == 0:
                        if kbias is None:
                            nc.scalar.activation(kt8[:, osl], pk[:], AF.Relu)
                        else:
                            nc.scalar.activation(kt8[:, osl], pk[:], AF.Relu,
                                                 bias=kbias)
                    else:
                        nc.vector.tensor_scalar(
                            out=kt8[:, osl], in0=pk[:],
                            scalar1=0.0 if zero_bias else kbias,
                            scalar2=0.0, op0=ALU.add, op1=ALU.max,
                        )

            # V -> fp8 pair-packs v8[a][p, i*H + h], m = a*256 + i*128 + p.
            # The two m-chunks of a pair share one [P, 512] psum, so one DVE
            # relu covers both.
            v8 = []
            for a in range(4):
                t = sb.tile([P, 2 * H], fp8, tag=f"v8_{a}", bufs=3,
                            name=f"v8_{a}_{b}")
                v8.append(t)
            for a in range(4):
                pv = ps.tile([P, 2 * H], f32, tag="sv", bufs=2,
                             name=f"pv{b}_{a}")
                for i in range(2):
                    mc = 2 * a + i
                    msl = slice(mc * P, (mc + 1) * P)
                    nc.tensor.matmul(
                        pv[:, i * H: (i + 1) * H], x83[:, :, msl],
                        wv3[:], start=True, stop=zero_bias,
                        perf_mode=DR, skip_group_check=True)
                    if not zero_bias:
                        nc.tensor.matmul(pv[:, i * H: (i + 1) * H],
                                         ones_row[:, 0:P], bv_sb[:],
                                         start=False, stop=True,
                                         skip_group_check=True)
                nc.vector.tensor_scalar_max(v8[a][:], pv[:], 0.0)
            st[b] = {"qt8": qt8, "kt8": kt8, "v8": v8}

        def s_phase(b):
            qt8, kt8 = st[b]["qt8"], st[b]["kt8"]
            qt3 = qt8[:].rearrange("p (i n) -> p i n", i=2)
            kt3 = kt8[:].rearrange("p (i n) -> p i n", i=2)
            i240a = i240[:, 0:256].rearrange("p (i m) -> p i m", i=2)
            i240b = i240[:, 256:512].rearrange("p (i m) -> p i m", i=2)
            ones3 = ones8.rearrange("p (i o) -> p i o", i=2)
            pmt = []
            for a in range(4):
                t = sb.tile([P, 2 * N], fp8, tag=f"pmt{a}", bufs=3,
                            name=f"pmt{a}_{b}")
                pmt.append(t)
            if b + 1 < bp:
                prefetch_phase(b + 1)
            mkg = st["mkg"].pop(b)
            mid = st.pop("mid", None)
            split = b == 0 or b == bp - 1
            if split:
                # first/last batch: nh-major order with half-size score
                # tiles.  All nh0 exps complete first, so batch 0's exps
                # start before the nh1 inputs even arrive, and the last
                # batch's nh0 AV/Y/output drain overlaps its nh1 exps.
                done = 0
                for nh in range(NHALF):
                    osl = slice(nh * 512, (nh + 1) * 512)
                    inj = i240a if nh == 0 else i240b
                    for mc in range(NSUB):
                        if done == 8 and mid is not None:
                            mid()
                            mid = None
                        mk3 = (mkg[mc // 4][:, (mc % 4) * N: (mc % 4 + 1) * N]
                               .rearrange("p (i n) -> p i n", i=2))
                        ssh = ps.tile([P, 512], f32, tag="ss", bufs=2,
                                      name=f"ssh{b}_{nh}_{mc}")
                        nc.tensor.matmul(ssh[:], inj, mk3,
                                         start=True, stop=False, perf_mode=DR)
                        nc.tensor.matmul(
                            ssh[:], kt3[:, :, mc * P: (mc + 1) * P],
                            qt3[:, :, osl],
                            start=False, stop=True, perf_mode=DR)
                        a, i = mc // 2, mc % 2
                        nc.scalar.activation(
                            pmt[a][:, i * N + nh * 512: i * N + nh * 512 + 512],
                            ssh[:], AF.Exp, bias=bias_sh[:])
                        done += 1
            else:
                for mc in range(NSUB):
                    if mc == 4 and mid is not None:
                        # emit the next batch's QKV mid-stream: its epilogues
                        # slot between this batch's exps on ACT/DVE, so the
                        # next S matmuls are unblocked the moment exp7 retires
                        mid()
                    mk3 = (mkg[mc // 4][:, (mc % 4) * N: (mc % 4 + 1) * N]
                           .rearrange("p (i n) -> p i n", i=2))
                    ss = ps.tile([P, N], f32, tag="ss", bufs=2,
                                 name=f"ss{b}_{mc}")
                    a, i = mc // 2, mc % 2
                    for nh in range(NHALF):
                        osl = slice(nh * 512, (nh + 1) * 512)
                        inj = i240a if nh == 0 else i240b
                        nc.tensor.matmul(ss[:, osl], inj, mk3,
                                         start=True, stop=False, perf_mode=DR)
                        nc.tensor.matmul(
                            ss[:, osl], kt3[:, :, mc * P: (mc + 1) * P],
                            qt3[:, :, osl],
                            start=False, stop=True, perf_mode=DR)
                    nc.scalar.activation(pmt[a][:, i * N: (i + 1) * N], ss[:],
                                         AF.Exp, bias=bias_sh[:])
            # row sums d land as PSUM columns via F=1 DoubleRow matmuls.
            # pd borrows the ss tag (uses its first 8 columns) so the sv tag
            # rotation never couples y epilogues to the next batch's exps.
            # column-group order matters: each d column's accumulation group
            # must COMPLETE before the next column's start=True — a start
            # marks its whole 2KB PSUM region pending-zero, which would
            # convert another in-flight group's accumulate into an overwrite
            pd = ps.tile([P, 512], f32, tag="s", bufs=2, name=f"pd{b}")
            p3s = [pmt[a][:].rearrange("p (i n) -> p i n", i=2)
                   for a in range(4)]
            for nct in range(NSUB):
                ncsl = slice(nct * P, (nct + 1) * P)
                for a in range(4):
                    nc.tensor.matmul(
                        pd[:, nct: nct + 1], p3s[a][:, :, ncsl], ones3,
                        start=(a == 0), stop=(a == 3), perf_mode=DR,
                        skip_group_check=True)
            st[b]["pmt"] = pmt
            invd = sb.tile([P, NSUB], f32, tag="ivd", bufs=2, name=f"ivd{b}")
            nc.vector.reciprocal(invd[:], pd[:, 0:NSUB])
            st[b]["invd"] = invd

        def av_phase(b):
            pmt, v8 = st[b]["pmt"], st[b]["v8"]
            ot = [
                sb.tile([P, N], bf16, tag=f"ot{hc}", bufs=3, name=f"ot{hc}_{b}")
                for hc in range(2)
            ]
            for nh in range(NHALF):
                nsl = slice(nh * 512, (nh + 1) * 512)
                for hc in range(2):
                    po = ps.tile([P, 512], f32, tag="sv", bufs=2,
                                 name=f"po{b}_{hc}_{nh}")
                    for a in range(4):
                        p3 = pmt[a][:].rearrange("p (i n) -> p i n", i=2)
                        v3 = v8[a][:].rearrange("p (i h) -> p i h", i=2)
                        nc.tensor.matmul(
                            po[:], v3[:, :, hc * P: (hc + 1) * P],
                            p3[:, :, nsl],
                            start=(a == 0), stop=(a == 3), perf_mode=DR)
                    # last batch: split copies across ACT/DVE so the drain
                    # chain parallelizes (nothing else queued then)
                    if b == bp - 1 and hc == 0:
                        nc.scalar.copy(ot[hc][:, nsl], po[:])
                    else:
                        nc.vector.tensor_copy(ot[hc][:, nsl], po[:])
            st[b]["ot"] = ot

        def y_phase(b):
            ot, invd = st[b]["ot"], st[b]["invd"]
            if not zero_bias:
                # d row for the bo bias: transpose invd's source d... general
                # path: recompute d = 1/invd is wasteful; instead pack d rows
                # via gpsimd DMA from a DVE copy of pd. Keep it simple: the
                # harness always has zero biases; general path adds d*bo via
                # K=1 matmuls from a flattened d-row.
                pdr = ps.tile([NSUB, P], f32, tag="sdr", bufs=2, name=f"pdr{b}")
                dcol = sb.tile([P, NSUB], f32, tag="dcol", bufs=2,
                               name=f"dcol{b}")
                nc.vector.reciprocal(dcol[:], invd[:])  # back to d
                id128 = st.setdefault("_id128", None)
                if id128 is None:
                    from concourse.masks import make_identity
                    id128 = const.tile([P, P], f32, tag="idf32", name="id_f32")
                    make_identity(nc, id128[:])
                    st["_id128"] = id128
                nc.tensor.transpose(pdr[:], dcol[:], id128[:])
                drow_pack = sb.tile([NSUB, P], bf16, tag="drow", bufs=2,
                                    name=f"drow{b}")
                nc.vector.tensor_copy(drow_pack[:], pdr[:])
                drow_flat = sb.tile([1, N], bf16, tag="drowf", bufs=2,
                                    name=f"drowf{b}")
                for nct in range(NSUB):
                    nc.gpsimd.dma_start(
                        drow_flat[:, nct * P: (nct + 1) * P],
                        drow_pack[nct: nct + 1, :])
            # y-group tiles: 4 n-chunks each, one output DMA per group
            for g in range(2):
                yg = sb.tile([P, 4 * DOUT], f32, tag="y", bufs=4,
                             name=f"y{b}_{g}")
                for cpair in range(2):
                    py = ps.tile([P, 2 * DOUT], f32,
                                 tag=("s" if b == bp - 1 else "sv"), bufs=2,
                                 name=f"py{b}_{g}_{cpair}")
                    for i in range(2):
                        nct = g * 4 + cpair * 2 + i
                        ncsl = slice(nct * P, (nct + 1) * P)
                        for hc in range(2):
                            nc.tensor.matmul(
                                py[:, i * DOUT: (i + 1) * DOUT],
                                ot[hc][:, ncsl],
                                wo_sb[:, hc * DOUT: (hc + 1) * DOUT],
                                start=(hc == 0), stop=(zero_bias and hc == 1),
                                skip_group_check=True,
                            )
                        if not zero_bias:
                            nc.tensor.matmul(
                                py[:, i * DOUT: (i + 1) * DOUT],
                                drow_flat[:, ncsl], bo_sb[:],
                                start=False, stop=True, skip_group_check=True)
                    for i in range(2):
                        nct = g * 4 + cpair * 2 + i
                        oslc = slice((cpair * 2 + i) * DOUT,
                                     (cpair * 2 + i + 1) * DOUT)
                        if b == bp - 1 and i == 0:
                            # last batch: relu(py * invd) on ACT via the
                            # per-partition scale port, parallel with DVE
                            nc.scalar.activation(
                                yg[:, oslc], py[:, i * DOUT: (i + 1) * DOUT],
                                AF.Relu, scale=invd[:, nct: nct + 1])
                        else:
                            nc.vector.tensor_scalar(
                                out=yg[:, oslc],
                                in0=py[:, i * DOUT: (i + 1) * DOUT],
                                scalar1=invd[:, nct: nct + 1],
                                scalar2=0.0, op0=ALU.mult, op1=ALU.max,
                            )
                    # one output DMA per pair (last batch: per chunk, for
                    # the shortest possible drain chain)
                    if b == bp - 1:
                        for i in range(2):
                            nct = g * 4 + cpair * 2 + i
                            nc.sync.dma_start(
                                out_d[b, nct * P: (nct + 1) * P, :],
                                yg[:, (cpair * 2 + i) * DOUT:
                                   (cpair * 2 + i + 1) * DOUT])
                    else:
                        nc.sync.dma_start(
                            out_d[b, g * 512 + cpair * 256:
                                  g * 512 + (cpair + 1) * 256, :]
                            .rearrange("(c p) o -> p c o", c=2),
                            yg[:, cpair * 2 * DOUT: (cpair * 2 + 2) * DOUT]
                            .rearrange("p (c o) -> p c o", c=2))
            del st[b]

        # PE warmup: keep the tensor engine continuously busy through the
        # initial DMA wait so the p-state ramp completes before real work
        warm_sb = const.tile([P, 64], bf16, tag="warm", name="warm_sb")
        nc.vector.memset(warm_sb[:], 0.0)
        warm_ps = ps.tile([P, N], f32, tag="ss", bufs=2, name="warm_ps")
        for w in range(110):
            nc.tensor.matmul(warm_ps[0:64, 0:64], warm_sb[:, 0:64],
                             warm_sb[:, 0:64], start=True, stop=True,
                             skip_group_check=True)

        # phase-interleaved emission: keep the PE fed with the next batch's
        # matmuls while ACT/DVE work through the current batch's epilogues
        prefetch_phase(0)
        qkv_phase(0)
        for b in range(bp):
            if b + 1 < bp:
                st["mid"] = (lambda bb: lambda: qkv_phase(bb))(b + 1)
            s_phase(b)
            # av/y of the PREVIOUS batch go after s(b): the next batch's S
            # matmuls outrank them so the exp stream never waits on the PE
            if b > 0:
                av_phase(b - 1)
                y_phase(b - 1)
        av_phase(bp - 1)
        y_phase(bp - 1)

    nc.compile()
    return nc


def _get_nc(bp=BP, zero_bias=True):
    key = (bp, zero_bias)
    if key not in _nc_cache:
        _nc_cache[key] = _build_nc(bp, zero_bias)
    return _nc_cache[key]


def kernel(x, mask, Wv, bv, Wk, bk, Wq, bq, Wo, bo):
    global last_results
    import ml_dtypes
    from concourse.bass_utils import run_bass_kernel_spmd

    bf = ml_dtypes.bfloat16
    e4 = ml_dtypes.float8_e4m3
    x = np.asarray(x, np.float32)
    xT = np.ascontiguousarray(x.transpose(0, 2, 1)).astype(bf)  # [B, D, N]
    xT8 = xT.astype(e4)
    # maskT - 1 in {-1, 0}, fp8; PE injects *240 into the score PSUM
    mkT8 = (np.asarray(mask, np.float32).transpose(0, 2, 1) - 1.0).astype(e4)
    mkT8 = np.ascontiguousarray(mkT8)

    idx = np.arange(P)
    kconst = np.zeros((P, 1026), e4)
    wk8 = np.asarray(Wk, np.float32).astype(bf).astype(e4)
    kconst[:, 0:512] = wk8.reshape(2, P, H).transpose(1, 0, 2).reshape(P, 512)
    kconst[idx, 512 + idx] = 240.0        # [240*I, 0]  (n-half 0)
    kconst[idx, 896 + idx] = 240.0        # [0, 240*I]  (n-half 1)
    kconst[:, 1024:1026] = 1.0

    w = {
        "Wq": np.ascontiguousarray(np.asarray(Wq, np.float32)).astype(bf),
        "Wk": np.ascontiguousarray(np.asarray(Wk, np.float32)).astype(bf),
        "Wv": np.ascontiguousarray(np.asarray(Wv, np.float32)).astype(bf),
        "kconst": kconst,
        "Wv8": np.ascontiguousarray(np.asarray(Wv, np.float32)).astype(bf).astype(e4),
        "Wo": np.ascontiguousarray(np.asarray(Wo, np.float32)).astype(bf),
        "bq": np.asarray(bq, np.float32).reshape(H, 1).copy(),
        "bk": np.asarray(bk, np.float32).reshape(H, 1).copy(),
        "bv": np.asarray(bv, np.float32).reshape(1, H).astype(bf),
        "bo": np.asarray(bo, np.float32).reshape(1, DOUT).astype(bf),
        "ones_row": np.ones((1, N), bf),
    }

    zero_bias = not (np.any(np.asarray(bq, np.float32))
                     or np.any(np.asarray(bk, np.float32))
                     or np.any(np.asarray(bv, np.float32))
                     or np.any(np.asarray(bo, np.float32)))
    nc = _get_nc(BP, zero_bias)
    in_maps = []
    for c in range(NCORES):
        sl = slice(c * BP, (c + 1) * BP)
        m = {"xT": np.ascontiguousarray(xT[sl]),
             "xT8": np.ascontiguousarray(xT8[sl]),
             "maskT8": np.ascontiguousarray(mkT8[sl])}
        m.update(w)
        in_maps.append(m)

    trace = bool(int(os.environ.get("BASS_KERNEL_TRACE", "0")))
    try:
        res = run_bass_kernel_spmd(
            nc, in_maps, core_ids=list(range(NCORES)), trace=trace
        )
    except Exception:
        if not trace:
            raise
        res = run_bass_kernel_spmd(nc, in_maps, core_ids=list(range(NCORES)))
    last_results = res
    out = np.concatenate([r["out"] for r in res.results], axis=0)
    return np.ascontiguousarray(out.astype(np.float32))


if __name__ == "__main__":
    nc = _get_nc(1)
    print("built ok:", nc)


# revision 57
# speedup vs baseline: 1.9799x; 1.0001x over previous
"""Trainium2 Bass kernel for nn_AttModel (masked GNN attention).

Reference computation (per batch b of 32, N=1024, D=H=O=256):
    v = relu(x @ Wv); q = relu(x @ Wq); k = relu(x @ Wk)      (biases zero)
    S = q @ k^T
    att = softmax(S * mask - 9e15 * (1 - mask), axis=-1)
    out = relu((att @ v) @ Wo)

Strategy: pure data parallelism over batch — 8 NeuronCores, 4 batches
each, weights replicated, no collectives.  Per batch, computed in the
TRANSPOSED score orientation S^T[m, n] so the post-softmax matrix feeds
the AV matmul directly (no PE transposes at all):

  - Q^T, K^T [h, n] = relu(W^T x^T) in bf16 matmuls, epilogues write
    fp8(e4m3) packs [p, hc, n]; V [m, h] epilogue writes fp8 packs
    [p, i, h] pairing m-chunks for DoubleRow.
  - Mask handling costs no vector-engine work: the host sends
    maskT-1 in {-1, 0} as fp8; the PE injects M = 240*(maskT-1) into
    PSUM via a DoubleRow matmul with stationary [240*I, 0] (or
    [0, 240*I] for the other n-half), then the S^T = K^T^T Q^T fp8
    DoubleRow matmuls accumulate on top.  Masked scores sit at ~-236,
    so exp underflows to exactly 0.
  - One ACT pass per m-pair does exp(S' - 6) straight from PSUM into
    fp8 P~^T tiles (the -6 shift keeps exp under fp8 e4m3 max 240 and
    cancels in the softmax normalization).
  - Row sums d (softmax denominators) come out as PSUM columns from
    F=1 DoubleRow matmuls with a ones vector: d[n-chunk] =
    (P~^T chunk)^T @ 1; reciprocal on DVE.
  - O^T = V^T P~^T via fp8 DoubleRow (K=256 per step), Y = O^T^T Wo in
    bf16, final epilogue fuses the deferred softmax normalization:
    y = max(py * (1/d), 0).

fp8(e4m3) is applied only where measured error allows: q, k, P~, v
values plus the K- and V-projection matmuls (DoubleRow, fp8 x and W);
the Q-projection and the output matmul stay bf16.  End-to-end error vs
the fp32 reference: fro 0.0149, absmax/scale 0.0169 (budget 2e-2).

Schedule: software-pipelined across the 4 batches (prefetch 2 ahead;
qkv(b+1) emitted mid-s(b) so its epilogues interleave between exps on
ACT/DVE; av/y of batch b-1 emitted after s(b) so next-batch S matmuls
outrank them on the PE).  PSUM tag layout (8 banks): ss 2x[128,1024]
(scores), s 2x[128,512] (q/k psums + d columns), sv 2x[128,512]
(V/AV/Y psums).  A PE warmup chain covers the initial DMA wait so the
p-state ramp finishes before real matmuls.  The first/last batches
run their score matmuls + exps in nh-major order with half-size score
tiles, shortening pipeline fill and drain; batch 0's x inputs arrive
in staged n-half DMAs and the fp8 constants (Wk8 pack, 240*I pair,
ones) ship as one host-prepacked DMA to cut head dispatch slots.
Cost-model (TimelineSim) exec estimate: 61.9 us/core (baseline bf16
kernel: 122.5 us).
"""

import os

import numpy as np

B, N, DIN, H, DOUT = 32, 1024, 256, 256, 256
NCORES = 8
BP = B // NCORES  # batches per core
P = 128
NSUB = N // P     # 8 row-chunks of 128
NHALF = N // 512  # 2 column-halves of 512
EXP_SHIFT = -6.0  # exp(S - 6): keeps exp(S) under fp8 max; cancels in softmax

_nc_cache = {}
last_results = None  # BassKernelResults of the most recent run (for test.py)


def _build_nc(bp=BP, zero_bias=True):
    import concourse.mybir as mybir
    import concourse.tile as tile
    from concourse import bacc
    from contextlib import ExitStack

    f32 = mybir.dt.float32
    bf16 = mybir.dt.bfloat16
    fp8 = mybir.dt.float8e4
    AF = mybir.ActivationFunctionType
    ALU = mybir.AluOpType
    DR = mybir.MatmulPerfMode.DoubleRow

    nc = bacc.Bacc("TRN2", target_bir_lowering=False)

    xT_d = nc.declare_dram_parameter("xT", [bp, DIN, N], bf16, isOutput=False)
    xT8_d = nc.declare_dram_parameter("xT8", [bp, DIN, N], fp8, isOutput=False)
    kc_d = nc.declare_dram_parameter("kconst", [P, 1026], fp8, isOutput=False)
    wv8_d = nc.declare_dram_parameter("Wv8", [DIN, H], fp8, isOutput=False)
    mask_d = nc.declare_dram_parameter("maskT8", [bp, N, N], fp8, isOutput=False)
    wq_d = nc.declare_dram_parameter("Wq", [DIN, H], bf16, isOutput=False)
    wk_d = nc.declare_dram_parameter("Wk", [DIN, H], bf16, isOutput=False)
    wv_d = nc.declare_dram_parameter("Wv", [DIN, H], bf16, isOutput=False)
    wo_d = nc.declare_dram_parameter("Wo", [H, DOUT], bf16, isOutput=False)
    bq_d = nc.declare_dram_parameter("bq", [H, 1], f32, isOutput=False)
    bk_d = nc.declare_dram_parameter("bk", [H, 1], f32, isOutput=False)
    bv_d = nc.declare_dram_parameter("bv", [1, H], bf16, isOutput=False)
    bo_d = nc.declare_dram_parameter("bo", [1, DOUT], bf16, isOutput=False)
    ones_row_d = nc.declare_dram_parameter("ones_row", [1, N], bf16,
                                           isOutput=False)
    out_d = nc.declare_dram_parameter("out", [bp, N, DOUT], f32, isOutput=True)

    with tile.TileContext(nc) as tc, ExitStack() as ctx:
        const = ctx.enter_context(tc.tile_pool(name="const", bufs=1))
        sb = ctx.enter_context(tc.tile_pool(name="sb", bufs=1))
        ps = ctx.enter_context(tc.tile_pool(name="ps", bufs=1, space="PSUM"))

        # ---- constants / weights (loaded once) ----
        # w*_pack[p, c*H + h] = W[c*P + p, h]
        def load_w(d, nm, width, dt=bf16):
            t = const.tile([P, 2 * width], dt, tag=nm, name=nm)
            nc.sync.dma_start(
                t[:].rearrange("p (c h) -> p c h", c=2),
                d.rearrange("(c p) h -> p c h", c=2))
            return t

        wq_sb = wk_sb = wv_sb = wo_sb = None  # loaded after batch-0 xt DMA
        # one host-prepacked fp8 const DMA: [wk8 pack | 240*I pair | ones]
        kconst = const.tile([P, 1026], fp8, tag="kc", name="kconst")
        i240 = kconst[:, 512:1024]
        ones8 = kconst[:, 1024:1026]
        bias_sh = const.tile([P, 1], f32, tag="bsh", name="bias_sh")

        if not zero_bias:
            def load_b(d, nm):
                t = const.tile([P, 2], f32, tag=nm, name=nm)
                nc.gpsimd.dma_start(
                    t[:].rearrange("p (c o) -> p c o", c=2),
                    d.rearrange("(c p) o -> p c o", c=2))
                return [t[:, c: c + 1] for c in range(2)]

            bq_sb = load_b(bq_d, "bqp")
            bk_sb = load_b(bk_d, "bkp")
            bv_sb = const.tile([1, H], bf16, tag="bv", name="bv_sb")
            nc.gpsimd.dma_start(bv_sb[:], bv_d[:, :])
            bo_sb = const.tile([1, DOUT], bf16, tag="bo", name="bo_sb")
            nc.gpsimd.dma_start(bo_sb[:], bo_d[:, :])
            ones_row = const.tile([1, N], bf16, tag="onesr", name="ones_row")
            nc.gpsimd.dma_start(ones_row[:], ones_row_d[:, :])
        else:
            bq_sb = bk_sb = bv_sb = bo_sb = ones_row = None

        # shifted exp bias per h-chunk for q/k epilogues (general path)
        st = {}

        def prefetch_phase(b):
            nonlocal wq_sb, wk_sb, wv_sb, wo_sb
            # one DMA for both d-chunks of x^T: xt_t[p, c*N + n] = xT[c*P+p, n]
            xt_t = sb.tile([P, 2 * N], bf16, tag="xt", bufs=4, name=f"xt_{b}")
            xt8_t = sb.tile([P, 2 * N], fp8, tag="xt8", bufs=4, name=f"xt8_{b}")
            x3 = xt_t[:].rearrange("p (c n) -> p c n", c=2)
            x3d = xT_d[b].rearrange("(c p) n -> p c n", c=2)
            x83 = xt8_t[:].rearrange("p (c n) -> p c n", c=2)
            x83d = xT8_d[b].rearrange("(c p) n -> p c n", c=2)
            mkg = [sb.tile([P, 4 * N], fp8, tag="mask", bufs=6,
                           name=f"mk{b}_{g}") for g in range(2)]

            def mask_dma(g):
                nc.sync.dma_start(
                    mkg[g][:].rearrange("p (c n) -> p c n", c=4),
                    mask_d[b, g * 512: (g + 1) * 512, :]
                    .rearrange("(c p) n -> p c n", c=4))

            if b == 0:
                # batch 0 is nh-major: phase A only touches the first
                # n-halves of x/x8, so stage the input DMAs accordingly
                nc.sync.dma_start(x3[:, :, 0:512], x3d[:, :, 0:512])
                nc.sync.dma_start(x83[:, :, 0:512], x83d[:, :, 0:512])
                nc.sync.dma_start(kconst[:], kc_d[:, :])
                wk_sb = kconst[:, 0:512]
                wq_sb = load_w(wq_d, "wqp", H)
                nc.vector.memset(bias_sh[:], EXP_SHIFT)
                mask_dma(0)
                nc.sync.dma_start(x3[:, :, 512:1024], x3d[:, :, 512:1024])
                nc.sync.dma_start(x83[:, :, 512:1024], x83d[:, :, 512:1024])
                mask_dma(1)
            else:
                nc.sync.dma_start(x3, x3d)
                nc.sync.dma_start(x83, x83d)
                mask_dma(0)
                mask_dma(1)
            st.setdefault("xt", {})[b] = xt_t
            st.setdefault("xt8", {})[b] = xt8_t
            st.setdefault("mkg", {})[b] = mkg
            if wv_sb is None:
                # after the masks: V/Y weights aren't needed until later
                wv_sb = load_w(wv8_d, "wvp", H, fp8)
                wo_sb = load_w(wo_d, "wop", DOUT)

        def qkv_phase(b):
            xt_t = st["xt"].pop(b)
            xt = [xt_t[:, c * N: (c + 1) * N] for c in range(2)]
            xt8_t = st["xt8"].pop(b)
            x83 = xt8_t[:].rearrange("p (c n) -> p c n", c=2)
            wk3 = wk_sb.rearrange("p (c h) -> p c h", c=2)
            wv3 = wv_sb[:].rearrange("p (c h) -> p c h", c=2)

            # Q^T, K^T -> fp8 packs [p, hc*N + n].  nh-outer so both h-chunks
            # of an n-half finish first; epilogues alternate ACT/DVE so the
            # two chunks the next exp needs land in parallel.
            qt8 = sb.tile([P, 2 * N], fp8, tag="qt8", bufs=3, name=f"qt8_{b}")
            kt8 = sb.tile([P, 2 * N], fp8, tag="kt8", bufs=3, name=f"kt8_{b}")
            for nh in range(NHALF):
                for hc in range(2):
                    nsl = slice(nh * 512, (nh + 1) * 512)
                    osl = slice(hc * N + nh * 512, hc * N + nh * 512 + 512)
                    pq = ps.tile([P, 512], f32, tag="s", bufs=2,
                                 name=f"pq{b}_{hc}_{nh}")
                    for dc in range(2):
                        nc.tensor.matmul(
                            pq[:],
                            wq_sb[:, dc * H + hc * P: dc * H + (hc + 1) * P],
                            xt[dc][:, nsl],
                            start=(dc == 0), stop=(dc == 1),
                        )
                    qbias = None if zero_bias else bq_sb[hc][:]
                    if hc == 0:
                        if qbias is None:
                            nc.scalar.activation(qt8[:, osl], pq[:], AF.Relu)
                        else:
                            nc.scalar.activation(qt8[:, osl], pq[:], AF.Relu,
                                                 bias=qbias)
                    else:
                        nc.vector.tensor_scalar(
                            out=qt8[:, osl], in0=pq[:],
                            scalar1=0.0 if zero_bias else qbias,
                            scalar2=0.0, op0=ALU.add, op1=ALU.max,
                        )
                    pk = ps.tile([P, 512], f32, tag="s", bufs=2,
                                 name=f"pk{b}_{hc}_{nh}")
                    nc.tensor.matmul(
                        pk[:], wk3[:, :, hc * P: (hc + 1) * P],
                        x83[:, :, nsl], start=True, stop=True, perf_mode=DR)
                    kbias = None if zero_bias else bk_sb[hc][:]
                    if hc == 1 and nh == 0:
                        if kbias is None:
                            nc.scalar.activation(kt8[:, osl], pk[:], AF.Relu)
                        else:
                            nc.scalar.activation(kt8[:, osl], pk[:], AF.Relu,
                                                 bias=kbias)
                    else:
                        nc.vector.tensor_scalar(
                            out=kt8[:, osl], in0=pk[:],
                            scalar1=0.0 if zero_bias else kbias,
                            scalar2=0.0, op0=ALU.add, op1=ALU.max,
                        )

            # V -> fp8 pair-packs v8[a][p, i*H + h], m = a*256 + i*128 + p.
            # The two m-chunks of a pair share one [P, 512] psum, so one DVE
            # relu covers both.
            v8 = []
            for a in range(4):
                t = sb.tile([P, 2 * H], fp8, tag=f"v8_{a}", bufs=3,
                            name=f"v8_{a}_{b}")
                v8.append(t)
            for a in range(4):
                pv = ps.tile([P, 2 * H], f32, tag="sv", bufs=2,
                             name=f"pv{b}_{a}")
                for i in range(2):
                    mc = 2 * a + i
                    msl = slice(mc * P, (mc + 1) * P)
                    nc.tensor.matmul(
                        pv[:, i * H: (i + 1) * H], x83[:, :, msl],
                        wv3[:], start=True, stop=zero_bias,
                        perf_mode=DR, skip_group_check=True)
                    if not zero_bias:
                        nc.tensor.matmul(pv[:, i * H: (i + 1) * H],
                                         ones_row[:, 0:P], bv_sb[:],
                                         start=False, stop=True,
                                         skip_group_check=True)
                nc.vector.tensor_scalar_max(v8[a][:], pv[:], 0.0)
            st[b] = {"qt8": qt8, "kt8": kt8, "v8": v8}

        def s_phase(b):
            qt8, kt8 = st[b]["qt8"], st[b]["kt8"]
            qt3 = qt8[:].rearrange("p (i n) -> p i n", i=2)
            kt3 = kt8[:].rearrange("p (i n) -> p i n", i=2)
            i240a = i240[:, 0:256].rearrange("p (i m) -> p i m", i=2)
            i240b = i240[:, 256:512].rearrange("p (i m) -> p i m", i=2)
            ones3 = ones8.rearrange("p (i o) -> p i o", i=2)
            pmt = []
            for a in range(4):
                t = sb.tile([P, 2 * N], fp8, tag=f"pmt{a}", bufs=3,
                            name=f"pmt{a}_{b}")
                pmt.append(t)
            if b + 1 < bp:
                prefetch_phase(b + 1)
            mkg = st["mkg"].pop(b)
            mid = st.pop("mid", None)
            split = b == 0 or b == bp - 1
            if split:
                # first/last batch: nh-major order with half-size score
                # tiles.  All nh0 exps complete first, so batch 0's exps
                # start before the nh1 inputs even arrive, and the last
                # batch's nh0 AV/Y/output drain overlaps its nh1 exps.
                done = 0
                for nh in range(NHALF):
                    osl = slice(nh * 512, (nh + 1) * 512)
                    inj = i240a if nh == 0 else i240b
                    for mc in range(NSUB):
                        if done == 8 and mid is not None:
                            mid()
                            mid = None
                        mk3 = (mkg[mc // 4][:, (mc % 4) * N: (mc % 4 + 1) * N]
                               .rearrange("p (i n) -> p i n", i=2))
                        ssh = ps.tile([P, 512], f32, tag="ss", bufs=2,
                                      name=f"ssh{b}_{nh}_{mc}")
                        nc.tensor.matmul(ssh[:], inj, mk3,
                                         start=True, stop=False, perf_mode=DR)
                        nc.tensor.matmul(
                            ssh[:], kt3[:, :, mc * P: (mc + 1) * P],
                            qt3[:, :, osl],
                            start=False, stop=True, perf_mode=DR)
                        a, i = mc // 2, mc % 2
                        nc.scalar.activation(
                            pmt[a][:, i * N + nh * 512: i * N + nh * 512 + 512],
                            ssh[:], AF.Exp, bias=bias_sh[:])
                        done += 1
            else:
                for mc in range(NSUB):
                    if mc == 4 and mid is not None:
                        # emit the next batch's QKV mid-stream: its epilogues
                        # slot between this batch's exps on ACT/DVE, so the
                        # next S matmuls are unblocked the moment exp7 retires
                        mid()
                    mk3 = (mkg[mc // 4][:, (mc % 4) * N: (mc % 4 + 1) * N]
                           .rearrange("p (i n) -> p i n", i=2))
                    ss = ps.tile([P, N], f32, tag="ss", bufs=2,
                                 name=f"ss{b}_{mc}")
                    a, i = mc // 2, mc % 2
                    for nh in range(NHALF):
                        osl = slice(nh * 512, (nh + 1) * 512)
                        inj = i240a if nh == 0 else i240b
                        nc.tensor.matmul(ss[:, osl], inj, mk3,
                                         start=True, stop=False, perf_mode=DR)
                        nc.tensor.matmul(
                            ss[:, osl], kt3[:, :, mc * P: (mc + 1) * P],
                            qt3[:, :, osl],
                            start=False, stop=True, perf_mode=DR)
                    nc.scalar.activation(pmt[a][:, i * N: (i + 1) * N], ss[:],
                                         AF.Exp, bias=bias_sh[:])
            # row sums d land as PSUM columns via F=1 DoubleRow matmuls.
            # pd borrows the ss tag (uses its first 8 columns) so the sv tag
            # rotation never couples y epilogues to the next batch's exps.
            # column-group order matters: each d column's accumulation group
            # must COMPLETE before the next column's start=True — a start
            # marks its whole 2KB PSUM region pending-zero, which would
            # convert another in-flight group's accumulate into an overwrite
            pd = ps.tile([P, 512], f32, tag="s", bufs=2, name=f"pd{b}")
            p3s = [pmt[a][:].rearrange("p (i n) -> p i n", i=2)
                   for a in range(4)]
            for nct in range(NSUB):
                ncsl = slice(nct * P, (nct + 1) * P)
                for a in range(4):
                    nc.tensor.matmul(
                        pd[:, nct: nct + 1], p3s[a][:, :, ncsl], ones3,
                        start=(a == 0), stop=(a == 3), perf_mode=DR,
                        skip_group_check=True)
            st[b]["pmt"] = pmt
            invd = sb.tile([P, NSUB], f32, tag="ivd", bufs=2, name=f"ivd{b}")
            nc.vector.reciprocal(invd[:], pd[:, 0:NSUB])
            st[b]["invd"] = invd

        def av_phase(b):
            pmt, v8 = st[b]["pmt"], st[b]["v8"]
            ot = [
                sb.tile([P, N], bf16, tag=f"ot{hc}", bufs=3, name=f"ot{hc}_{b}")
                for hc in range(2)
            ]
            for nh in range(NHALF):
                nsl = slice(nh * 512, (nh + 1) * 512)
                for hc in range(2):
                    po = ps.tile([P, 512], f32, tag="sv", bufs=2,
                                 name=f"po{b}_{hc}_{nh}")
                    for a in range(4):
                        p3 = pmt[a][:].rearrange("p (i n) -> p i n", i=2)
                        v3 = v8[a][:].rearrange("p (i h) -> p i h", i=2)
                        nc.tensor.matmul(
                            po[:], v3[:, :, hc * P: (hc + 1) * P],
                            p3[:, :, nsl],
                            start=(a == 0), stop=(a == 3), perf_mode=DR)
                    # last batch: split copies across ACT/DVE so the drain
                    # chain parallelizes (nothing else queued then)
                    if b == bp - 1 and hc == 0:
                        nc.scalar.copy(ot[hc][:, nsl], po[:])
                    else:
                        nc.vector.tensor_copy(ot[hc][:, nsl], po[:])
            st[b]["ot"] = ot

        def y_phase(b):
            ot, invd = st[b]["ot"], st[b]["invd"]
            if not zero_bias:
                # d row for the bo bias: transpose invd's source d... general
                # path: recompute d = 1/invd is wasteful; instead pack d rows
                # via gpsimd DMA from a DVE copy of pd. Keep it simple: the
                # harness always has zero biases; general path adds d*bo via
                # K=1 matmuls from a flattened d-row.
                pdr = ps.tile([NSUB, P], f32, tag="sdr", bufs=2, name=f"pdr{b}")
                dcol = sb.tile([P, NSUB], f32, tag="dcol", bufs=2,
                               name=f"dcol{b}")
                nc.vector.reciprocal(dcol[:], invd[:])  # back to d
                id128 = st.setdefault("_id128", None)
                if id128 is None:
                    from concourse.masks import make_identity
                    id128 = const.tile([P, P], f32, tag="idf32", name="id_f32")
                    make_identity(nc, id128[:])
                    st["_id128"] = id128
                nc.tensor.transpose(pdr[:], dcol[:], id128[:])
                drow_pack = sb.tile([NSUB, P], bf16, tag="drow", bufs=2,
                                    name=f"drow{b}")
                nc.vector.tensor_copy(drow_pack[:], pdr[:])
                drow_flat = sb.tile([1, N], bf16, tag="drowf", bufs=2,
                                    name=f"drowf{b}")
                for nct in range(NSUB):
                    nc.gpsimd.dma_start(
                        drow_flat[:, nct * P: (nct + 1) * P],
                        drow_pack[nct: nct + 1, :])
            # y-group tiles: 4 n-chunks each, one output DMA per group
            for g in range(2):
                yg = sb.tile([P, 4 * DOUT], f32, tag="y", bufs=4,
                             name=f"y{b}_{g}")
                for cpair in range(2):
                    py = ps.tile([P, 2 * DOUT], f32,
                                 tag=("s" if b == bp - 1 else "sv"), bufs=2,
                                 name=f"py{b}_{g}_{cpair}")
                    for i in range(2):
                        nct = g * 4 + cpair * 2 + i
                        ncsl = slice(nct * P, (nct + 1) * P)
                        for hc in range(2):
                            nc.tensor.matmul(
                                py[:, i * DOUT: (i + 1) * DOUT],
                                ot[hc][:, ncsl],
                                wo_sb[:, hc * DOUT: (hc + 1) * DOUT],
                                start=(hc == 0), stop=(zero_bias and hc == 1),
                                skip_group_check=True,
                            )
                        if not zero_bias:
                            nc.tensor.matmul(
                                py[:, i * DOUT: (i + 1) * DOUT],
                                drow_flat[:, ncsl], bo_sb[:],
                                start=False, stop=True, skip_group_check=True)
                    for i in range(2):
                        nct = g * 4 + cpair * 2 + i
                        oslc = slice((cpair * 2 + i) * DOUT,
                                     (cpair * 2 + i + 1) * DOUT)
                        if b == bp - 1 and i == 0:
                            # last batch: relu(py * invd) on ACT via the
                            # per-partition scale port, parallel with DVE
                            nc.scalar.activation(
                                yg[:, oslc], py[:, i * DOUT: (i + 1) * DOUT],
                                AF.Relu, scale=invd[:, nct: nct + 1])
                        else:
                            nc.vector.tensor_scalar(
                                out=yg[:, oslc],
                                in0=py[:, i * DOUT: (i + 1) * DOUT],
                                scalar1=invd[:, nct: nct + 1],
                                scalar2=0.0, op0=ALU.mult, op1=ALU.max,
                            )
                    # one output DMA per pair (last batch: per chunk, for
                    # the shortest possible drain chain)
                    if b == bp - 1:
                        for i in range(2):
                            nct = g * 4 + cpair * 2 + i
                            nc.sync.dma_start(
                                out_d[b, nct * P: (nct + 1) * P, :],
                                yg[:, (cpair * 2 + i) * DOUT:
                                   (cpair * 2 + i + 1) * DOUT])
                    else:
                        nc.sync.dma_start(
                            out_d[b, g * 512 + cpair * 256:
                                  g * 512 + (cpair + 1) * 256, :]
                            .rearrange("(c p) o -> p c o", c=2),
                            yg[:, cpair * 2 * DOUT: (cpair * 2 + 2) * DOUT]
                            .rearrange("p (c o) -> p c o", c=2))
            del st[b]

        # PE warmup: keep the tensor engine continuously busy through the
        # initial DMA wait so the p-state ramp completes before real work
        warm_sb = const.tile([P, 64], bf16, tag="warm", name="warm_sb")
        nc.vector.memset(warm_sb[:], 0.0)
        warm_ps = ps.tile([P, N], f32, tag="ss", bufs=2, name="warm_ps")
        for w in range(110):
            nc.tensor.matmul(warm_ps[0:64, 0:64], warm_sb[:, 0:64],
                             warm_sb[:, 0:64], start=True, stop=True,
                             skip_group_check=True)

        # phase-interleaved emission: keep the PE fed with the next batch's
        # matmuls while ACT/DVE work through the current batch's epilogues
        prefetch_phase(0)
        qkv_phase(0)
        for b in range(bp):
            if b + 1 < bp:
                st["mid"] = (lambda bb: lambda: qkv_phase(bb))(b + 1)
            s_phase(b)
            # av/y of the PREVIOUS batch go after s(b): the next batch's S
            # matmuls outrank them so the exp stream never waits on the PE
            if b > 0:
                av_phase(b - 1)
                y_phase(b - 1)
        av_phase(bp - 1)
        y_phase(bp - 1)

    nc.compile()
    return nc


def _get_nc(bp=BP, zero_bias=True):
    key = (bp, zero_bias)
    if key not in _nc_cache:
        _nc_cache[key] = _build_nc(bp, zero_bias)
    return _nc_cache[key]


def kernel(x, mask, Wv, bv, Wk, bk, Wq, bq, Wo, bo):
    global last_results
    import ml_dtypes
    from concourse.bass_utils import run_bass_kernel_spmd

    bf = ml_dtypes.bfloat16
    e4 = ml_dtypes.float8_e4m3
    x = np.asarray(x, np.float32)
    xT = np.ascontiguousarray(x.transpose(0, 2, 1)).astype(bf)  # [B, D, N]
    xT8 = xT.astype(e4)
    # maskT - 1 in {-1, 0}, fp8; PE injects *240 into the score PSUM
    mkT8 = (np.asarray(mask, np.float32).transpose(0, 2, 1) - 1.0).astype(e4)
    mkT8 = np.ascontiguousarray(mkT8)

    idx = np.arange(P)
    kconst = np.zeros((P, 1026), e4)
    wk8 = np.asarray(Wk, np.float32).astype(bf).astype(e4)
    kconst[:, 0:512] = wk8.reshape(2, P, H).transpose(1, 0, 2).reshape(P, 512)
    kconst[idx, 512 + idx] = 240.0        # [240*I, 0]  (n-half 0)
    kconst[idx, 896 + idx] = 240.0        # [0, 240*I]  (n-half 1)
    kconst[:, 1024:1026] = 1.0

    w = {
        "Wq": np.ascontiguousarray(np.asarray(Wq, np.float32)).astype(bf),
        "Wk": np.ascontiguousarray(np.asarray(Wk, np.float32)).astype(bf),
        "Wv": np.ascontiguousarray(np.asarray(Wv, np.float32)).astype(bf),
        "kconst": kconst,
        "Wv8": np.ascontiguousarray(np.asarray(Wv, np.float32)).astype(bf).astype(e4),
        "Wo": np.ascontiguousarray(np.asarray(Wo, np.float32)).astype(bf),
        "bq": np.asarray(bq, np.float32).reshape(H, 1).copy(),
        "bk": np.asarray(bk, np.float32).reshape(H, 1).copy(),
        "bv": np.asarray(bv, np.float32).reshape(1, H).astype(bf),
        "bo": np.asarray(bo, np.float32).reshape(1, DOUT).astype(bf),
        "ones_row": np.ones((1, N), bf),
    }

    zero_bias = not (np.any(np.asarray(bq, np.float32))
                     or np.any(np.asarray(bk, np.float32))
                     or np.any(np.asarray(bv, np.float32))
                     or np.any(np.asarray(bo, np.float32)))
    nc = _get_nc(BP, zero_bias)
    in_maps = []
    for c in range(NCORES):
        sl = slice(c * BP, (c + 1) * BP)
        m = {"xT": np.ascontiguousarray(xT[sl]),
             "xT8": np.ascontiguousarray(xT8[sl]),
             "maskT8": np.ascontiguousarray(mkT8[sl])}
        m.update(w)
        in_maps.append(m)

    trace = bool(int(os.environ.get("BASS_KERNEL_TRACE", "0")))
    try:
        res = run_bass_kernel_spmd(
            nc, in_maps, core_ids=list(range(NCORES)), trace=trace
        )
    except Exception:
        if not trace:
            raise
        res = run_bass_kernel_spmd(nc, in_maps, core_ids=list(range(NCORES)))
    last_results = res
    out = np.concatenate([r["out"] for r in res.results], axis=0)
    return np.ascontiguousarray(out.astype(np.float32))


if __name__ == "__main__":
    nc = _get_nc(1)
    print("built ok:", nc)
